# revision 1
# baseline (speedup 1.0000x reference)
"""Deformable RoI pooling (deform_psroi_pooling, group_size=1) on 8 Trainium2
NeuronCores via Bass/Tile.

Strategy
--------
The reference computes, per roi r and output bin (ph, pw):

    out[r, c, ph, pw] = (1/max(cnt,1)) * sum_{valid samples s} bilinear(data[b_r, c], pos_s)

Every sample contributes 4 corner taps with weights independent of the
channel c.  Folding the bilinear weights, validity masking and the 1/cnt
normalisation together, each roi's output is a small matmul

    out[r, :, bin] = sum_{cells q} S_r[q, bin] * F[b_r, :, q]

with S_r a sparse per-roi weight matrix over the feature-map cells the roi
touches (computed on host in float32, exactly mirroring the reference
arithmetic), and F the feature map.

Device work per core (SPMD, one program, 8 cores):
  * feature map shipped channel-last as quad-cell rows [15200, 1024] f32
    (4 consecutive cells x 256 channels = 4KB per row),
  * indirect-DMA gathers of 128 quad-rows per instruction (one row per
    SBUF partition) pull exactly the cells its rois touch,
  * per 128-quad slot, 4 matmuls (lhsT = S slice [128,49], rhs = gathered
    channels [128,256]) accumulate into a [49, 256] PSUM tile per roi,
  * PSUM -> SBUF copy -> HBM out [49, NROI*256].

RoIs are dealt to cores of their image (cores 0-3 image 0, 4-7 image 1),
sorted by size and snake-dealt so all 8 cores execute an identical slot
profile; padding slots gather row 0 with all-zero S.
"""

import hashlib

import numpy as np

P = 7          # pooled size (== part size)
SPP = 4        # samples per part
SPATIAL_SCALE = np.float32(0.0625)
TRANS_STD = np.float32(0.1)
N_IMG, C_FEAT, H_FEAT, W_FEAT = 2, 256, 200, 304
QUAD = 4                      # cells per gathered row
NQROWS = H_FEAT * W_FEAT // QUAD          # 15200 quad rows per image
ROW_ELEMS = QUAD * C_FEAT                 # 1024 f32 per quad row
NBINS = P * P                             # 49
N_CORES = 8
SLOT_PAIRS = 128                          # quads per slot (one per partition)

_f32 = np.float32


def _host_tables(rois: np.ndarray, offset: np.ndarray):
    """Mirror the reference position math bit-exactly in float32 and build,
    per roi: the sorted list of quad-row ids it touches and the dense weight
    matrix S [nquads*4cells, 49] (weights already divided by max(cnt,1))."""
    R = rois.shape[0]
    rois = rois.astype(np.float32, copy=False)
    offset = offset.astype(np.float32, copy=False)

    b = rois[:, 0].astype(np.int32)
    roi_start_w = np.round(rois[:, 1]) * SPATIAL_SCALE - _f32(0.5)
    roi_start_h = np.round(rois[:, 2]) * SPATIAL_SCALE - _f32(0.5)
    roi_end_w = (np.round(rois[:, 3]) + _f32(1.0)) * SPATIAL_SCALE - _f32(0.5)
    roi_end_h = (np.round(rois[:, 4]) + _f32(1.0)) * SPATIAL_SCALE - _f32(0.5)
    roi_w = np.maximum(roi_end_w - roi_start_w, _f32(0.1))
    roi_h = np.maximum(roi_end_h - roi_start_h, _f32(0.1))
    bin_w = roi_w / _f32(P)
    bin_h = roi_h / _f32(P)
    sub_w = bin_w / _f32(SPP)
    sub_h = bin_h / _f32(SPP)

    ph = np.arange(P, dtype=np.float32)
    pw = np.arange(P, dtype=np.float32)
    # part_h == ph, part_w == pw for PART == P
    tx = offset[:, 0] * TRANS_STD                       # [R, P, P]
    ty = offset[:, 1] * TRANS_STD

    wstart = (pw[None, None, :] * bin_w[:, None, None]
              + roi_start_w[:, None, None] + tx * roi_w[:, None, None])
    hstart = (ph[None, :, None] * bin_h[:, None, None]
              + roi_start_h[:, None, None] + ty * roi_h[:, None, None])

    s = np.arange(SPP, dtype=np.float32)
    wpos = wstart[..., None, None] + s[None, None, None, None, :] * sub_w[:, None, None, None, None]
    hpos = hstart[..., None, None] + s[None, None, None, :, None] * sub_h[:, None, None, None, None]

    W = W_FEAT
    H = H_FEAT
    valid = ((wpos > _f32(-0.5)) & (wpos < _f32(W) - _f32(0.5))
             & (hpos > _f32(-0.5)) & (hpos < _f32(H) - _f32(0.5)))
    wc = np.clip(wpos, _f32(0.0), _f32(W - 1.0))
    hc = np.clip(hpos, _f32(0.0), _f32(H - 1.0))
    x0 = np.floor(wc)
    y0 = np.floor(hc)
    dx = wc - x0
    dy = hc - y0
    x0i = x0.astype(np.int32)
    y0i = y0.astype(np.int32)
    x1i = np.minimum(x0i + 1, W - 1)
    y1i = np.minimum(y0i + 1, H - 1)

    cnt = valid.sum(axis=(-1, -2)).astype(np.float32)           # [R, P, P]
    inv = _f32(1.0) / np.maximum(cnt, _f32(1.0))

    one = _f32(1.0)
    w00 = (one - dx) * (one - dy)
    w01 = dx * (one - dy)
    w10 = (one - dx) * dy
    w11 = dx * dy

    bins = np.broadcast_to(
        (np.arange(P)[:, None] * P + np.arange(P)[None, :])[None, :, :, None, None],
        valid.shape,
    )
    scale = np.broadcast_to(inv[:, :, :, None, None], valid.shape)

    per_roi = []
    for r in range(R):
        v = valid[r].ravel()
        if not v.any():
            per_roi.append((int(b[r]), np.zeros(1, np.int32),
                            np.zeros((1, QUAD, NBINS), np.float32)))
            continue
        shp = valid[r].shape
        bc = lambda a: np.broadcast_to(a, shp).ravel()[v]
        sc = bc(scale[r]).astype(np.float32)
        bn = bc(bins[r]).astype(np.int64)
        cy0 = bc(y0i[r]).astype(np.int64)
        cy1 = bc(y1i[r]).astype(np.int64)
        cx0 = bc(x0i[r]).astype(np.int64)
        cx1 = bc(x1i[r]).astype(np.int64)
        ws = [bc(w00[r]) * sc, bc(w01[r]) * sc,
              bc(w10[r]) * sc, bc(w11[r]) * sc]
        cells = [cy0 * W + cx0, cy0 * W + cx1, cy1 * W + cx0, cy1 * W + cx1]

        cell_all = np.concatenate(cells)
        w_all = np.concatenate(ws).astype(np.float64)
        bin_all = np.concatenate([bn] * 4)

        quads = np.unique(cell_all >> 2).astype(np.int32)       # sorted
        qpos = np.searchsorted(quads, cell_all >> 2)
        key = (qpos * QUAD + (cell_all & 3)) * NBINS + bin_all
        S = np.bincount(key, weights=w_all,
                        minlength=len(quads) * QUAD * NBINS)
        S = S.astype(np.float32).reshape(len(quads), QUAD, NBINS)
        per_roi.append((int(b[r]), quads, S))
    return per_roi


def _deal_to_cores(per_roi):
    """Assign rois to cores (cores 0-3 image 0, 4-7 image 1) snake-dealt by
    descending chunk count; build the shared slot profile."""
    img_rois = {0: [], 1: []}
    for rid, (img, quads, S) in enumerate(per_roi):
        nchunk = (len(quads) + SLOT_PAIRS - 1) // SLOT_PAIRS
        img_rois[img].append((nchunk, rid))
    core_rois = [[] for _ in range(N_CORES)]
    for img, lst in img_rois.items():
        lst.sort(reverse=True)
        cores = list(range(4 * img, 4 * img + 4))
        for i, item in enumerate(lst):
            k = i % 8
            c = cores[k] if k < 4 else cores[7 - k]
            core_rois[c].append(item)
    for c in range(N_CORES):
        core_rois[c].sort(reverse=True)          # descending chunk count
    nroi = max(1, max(len(cr) for cr in core_rois))
    profile = []
    for k in range(nroi):
        profile.append(max((cr[k][0] if k < len(cr) else 1)
                           for cr in core_rois))
    return core_rois, tuple(profile)


_PROGRAM_CACHE: dict = {}


def _build_program(profile):
    """One SPMD Tile program for all 8 cores, parameterised only by the slot
    profile (chunks per roi slot)."""
    key = profile
    if key in _PROGRAM_CACHE:
        return _PROGRAM_CACHE[key]

    from concourse import bass, mybir, bacc
    from concourse.tile import TileContext

    nroi = len(profile)
    nslot = sum(profile)

    nc = bacc.Bacc("TRN2", target_bir_lowering=False, debug=False,
                   num_devices=N_CORES)
    dataT = nc.declare_dram_parameter("dataT", [NQROWS, ROW_ELEMS],
                                      mybir.dt.float32, isOutput=False)
    offs = nc.declare_dram_parameter("offs", [128, nslot],
                                     mybir.dt.int32, isOutput=False)
    spack = nc.declare_dram_parameter("spack", [128, nslot * QUAD * NBINS],
                                      mybir.dt.float32, isOutput=False)
    out = nc.declare_dram_parameter("out", [NBINS, nroi * C_FEAT],
                                    mybir.dt.float32, isOutput=True)

    with TileContext(nc) as tc:
        with (
            tc.tile_pool(name="const", bufs=1) as cpool,
            tc.tile_pool(name="gt", bufs=6) as gpool,
            tc.tile_pool(name="ps", bufs=4, space="PSUM") as pspool,
            tc.tile_pool(name="ob", bufs=4) as opool,
        ):
            offs_t = cpool.tile([128, nslot], mybir.dt.int32)
            nc.sync.dma_start(out=offs_t[:], in_=offs[:])
            s_t = cpool.tile([128, nslot * QUAD * NBINS], mybir.dt.float32)
            # Load S in chunks so early matmuls can start sooner.
            scols = nslot * QUAD * NBINS
            nq = 8
            for q in range(nq):
                lo = q * scols // nq
                hi = (q + 1) * scols // nq
                nc.sync.dma_start(out=s_t[:, lo:hi], in_=spack[:, lo:hi])

            slot = 0
            for k in range(nroi):
                ps = pspool.tile([NBINS, C_FEAT], mybir.dt.float32)
                nch = profile[k]
                for j in range(nch):
                    gt = gpool.tile([128, ROW_ELEMS], mybir.dt.float32)
                    nc.gpsimd.indirect_dma_start(
                        out=gt[:],
                        out_offset=None,
                        in_=dataT[:],
                        in_offset=bass.IndirectOffsetOnAxis(
                            ap=offs_t[:, slot:slot + 1], axis=0),
                    )
                    for e in range(QUAD):
                        nc.tensor.matmul(
                            ps[:],
                            lhsT=s_t[:, (slot * QUAD + e) * NBINS:
                                     (slot * QUAD + e + 1) * NBINS],
                            rhs=gt[:, e * C_FEAT:(e + 1) * C_FEAT],
                            start=(j == 0 and e == 0),
                            stop=(j == nch - 1 and e == QUAD - 1),
                        )
                    slot += 1
                ob = opool.tile([NBINS, C_FEAT], mybir.dt.float32)
                nc.vector.tensor_copy(out=ob[:], in_=ps[:])
                nc.sync.dma_start(out=out[:, k * C_FEAT:(k + 1) * C_FEAT],
                                  in_=ob[:])
    nc.compile()
    _PROGRAM_CACHE[key] = nc
    return nc


def _core_inputs(per_roi, core_rois, profile, dataT_imgs):
    nroi = len(profile)
    nslot = sum(profile)
    base = np.cumsum([0] + list(profile))
    in_maps = []
    roi_of_slotk = []                      # per core: slot k -> roi id
    for c in range(N_CORES):
        img = 0 if c < 4 else 1
        offs = np.zeros((128, nslot), np.int32)
        spack = np.zeros((128, nslot * QUAD * NBINS), np.float32)
        rmap = [-1] * nroi
        for k, (nchunk, rid) in enumerate(core_rois[c]):
            rmap[k] = rid
            _, quads, S = per_roi[rid]
            npad = nchunk * SLOT_PAIRS
            qpad = np.zeros(npad, np.int32)
            qpad[:len(quads)] = quads
            Spad = np.zeros((npad, QUAD, NBINS), np.float32)
            Spad[:len(quads)] = S
            for j in range(nchunk):
                s0 = base[k] + j
                offs[:, s0] = qpad[j * 128:(j + 1) * 128]
                blk = Spad[j * 128:(j + 1) * 128]           # [128, 4, 49]
                spack[:, s0 * QUAD * NBINS:(s0 + 1) * QUAD * NBINS] = \
                    blk.reshape(128, QUAD * NBINS)
        in_maps.append({"dataT": dataT_imgs[img], "offs": offs,
                        "spack": spack})
        roi_of_slotk.append(rmap)
    return in_maps, roi_of_slotk


def kernel(data: np.ndarray, rois: np.ndarray, offset: np.ndarray) -> np.ndarray:
    from concourse.bass_utils import run_bass_kernel_spmd

    data = np.ascontiguousarray(data, dtype=np.float32)
    rois = np.asarray(rois, dtype=np.float32)
    offset = np.asarray(offset, dtype=np.float32)
    R = rois.shape[0]

    per_roi = _host_tables(rois, offset)
    core_rois, profile = _deal_to_cores(per_roi)
    nc = _build_program(profile)

    # channel-last quad-row layout per image: [15200, 1024] f32
    dataT_imgs = [
        np.ascontiguousarray(data[i].transpose(1, 2, 0)).reshape(NQROWS, ROW_ELEMS)
        for i in range(N_IMG)
    ]
    in_maps, roi_of_slotk = _core_inputs(per_roi, core_rois, profile, dataT_imgs)

    res = run_bass_kernel_spmd(nc, in_maps, list(range(N_CORES)), trace=False)

    out_full = np.zeros((R, C_FEAT, P, P), np.float32)
    nroi = len(profile)
    for c in range(N_CORES):
        o = np.asarray(res.results[c]["out"])          # [49, nroi*256]
        o = o.reshape(NBINS, nroi, C_FEAT).transpose(1, 2, 0)   # [nroi,256,49]
        for k, rid in enumerate(roi_of_slotk[c]):
            if rid >= 0:
                out_full[rid] = o[k].reshape(C_FEAT, P, P)
    return out_full



# revision 8
# speedup vs baseline: 1.0856x; 1.0856x over previous
"""Deformable RoI pooling (deform_psroi_pooling, group_size=1) on 8 Trainium2
NeuronCores via Bass/Tile.

Strategy
--------
Per roi r and output bin (ph, pw) the reference computes a weighted sum of
bilinear taps over feature-map cells; folding bilinear weights, validity
masking and 1/cnt normalisation gives a per-roi sparse matmul

    out[r, :, bin] = sum_{cells q} S_r[q, bin] * F[b_r, :, q]

Device work per core (SPMD, one program, 8 cores):
  * feature map shipped channel-last as pair-of-cell rows [30400, 512] bf16
    (1 KiB per row; pair ids fit dma_gather's int16 indices),
  * rois dealt to the 4 cores of their image, each roi's touched-pair list
    packed back-to-back at 64-row alignment into a canonical row layout
    shared by all 8 cores (~10% padding vs 45% for per-roi slot round-up),
  * dma_gather (InstDMAGatherAnt) fetches NSG*128 pair-rows per
    instruction -- index i lands at partition i%128, free chunk i//128 --
    paying the ~1us SWDGE fixed cost once per NSG slots,
  * per 128-row slot and roi-segment, 2 bf16 matmuls (one per cell of the
    pair; partition sub-ranges at legal PE bases 0/64) accumulate into the
    roi's [49, 256] PSUM tile,
  * PSUM -> SBUF bf16 downcast on the scalar (Act) engine -> HBM out
    [49, K*256] bf16; host upcasts.

bf16 data/weights halve HBM traffic and run the PE at 1 cycle/row
(fp32 is 4); PSUM accumulation stays fp32, keeping global rel err ~5e-3.
"""

import numpy as np

P = 7          # pooled size (== part size)
SPP = 4        # samples per part
SPATIAL_SCALE = np.float32(0.0625)
TRANS_STD = np.float32(0.1)
N_IMG, C_FEAT, H_FEAT, W_FEAT = 2, 256, 200, 304
CELLS = 2                                 # cells per gathered row (pair)
NROWSP = H_FEAT * W_FEAT // CELLS         # 30400 pair rows per image
ELEM = CELLS * C_FEAT                     # 512 bf16 elements per row (1 KiB)
NBINS = P * P                             # 49
N_CORES = 8
ALIGN = 64                                # roi packing alignment (PE tile rows;
                                          # AP base partition must be 0/32/64)
NSG = 8                                   # slots per dma_gather instruction

_f32 = np.float32


def _host_tables(rois: np.ndarray, offset: np.ndarray):
    """Mirror the reference position math bit-exactly in float32; per roi
    return (image, sorted touched pair-row ids, dense S [nrows, 2*49] with
    weights already divided by max(cnt, 1))."""
    R = rois.shape[0]
    rois = rois.astype(np.float32, copy=False)
    offset = offset.astype(np.float32, copy=False)

    b = rois[:, 0].astype(np.int32)
    roi_start_w = np.round(rois[:, 1]) * SPATIAL_SCALE - _f32(0.5)
    roi_start_h = np.round(rois[:, 2]) * SPATIAL_SCALE - _f32(0.5)
    roi_end_w = (np.round(rois[:, 3]) + _f32(1.0)) * SPATIAL_SCALE - _f32(0.5)
    roi_end_h = (np.round(rois[:, 4]) + _f32(1.0)) * SPATIAL_SCALE - _f32(0.5)
    roi_w = np.maximum(roi_end_w - roi_start_w, _f32(0.1))
    roi_h = np.maximum(roi_end_h - roi_start_h, _f32(0.1))
    bin_w = roi_w / _f32(P)
    bin_h = roi_h / _f32(P)
    sub_w = bin_w / _f32(SPP)
    sub_h = bin_h / _f32(SPP)

    ph = np.arange(P, dtype=np.float32)
    pw = np.arange(P, dtype=np.float32)
    tx = offset[:, 0] * TRANS_STD                       # [R, P, P]
    ty = offset[:, 1] * TRANS_STD

    wstart = (pw[None, None, :] * bin_w[:, None, None]
              + roi_start_w[:, None, None] + tx * roi_w[:, None, None])
    hstart = (ph[None, :, None] * bin_h[:, None, None]
              + roi_start_h[:, None, None] + ty * roi_h[:, None, None])

    s = np.arange(SPP, dtype=np.float32)
    wpos = wstart[..., None, None] + s[None, None, None, None, :] * sub_w[:, None, None, None, None]
    hpos = hstart[..., None, None] + s[None, None, None, :, None] * sub_h[:, None, None, None, None]

    W = W_FEAT
    H = H_FEAT
    valid = ((wpos > _f32(-0.5)) & (wpos < _f32(W) - _f32(0.5))
             & (hpos > _f32(-0.5)) & (hpos < _f32(H) - _f32(0.5)))
    wc = np.clip(wpos, _f32(0.0), _f32(W - 1.0))
    hc = np.clip(hpos, _f32(0.0), _f32(H - 1.0))
    x0 = np.floor(wc)
    y0 = np.floor(hc)
    dx = wc - x0
    dy = hc - y0
    x0i = x0.astype(np.int32)
    y0i = y0.astype(np.int32)
    x1i = np.minimum(x0i + 1, W - 1)
    y1i = np.minimum(y0i + 1, H - 1)

    cnt = valid.sum(axis=(-1, -2)).astype(np.float32)           # [R, P, P]
    inv = _f32(1.0) / np.maximum(cnt, _f32(1.0))

    one = _f32(1.0)
    w00 = (one - dx) * (one - dy)
    w01 = dx * (one - dy)
    w10 = (one - dx) * dy
    w11 = dx * dy

    bins = np.broadcast_to(
        (np.arange(P)[:, None] * P + np.arange(P)[None, :])[None, :, :, None, None],
        valid.shape,
    )
    scale = np.broadcast_to(inv[:, :, :, None, None], valid.shape)

    per_roi = []
    for r in range(R):
        v = valid[r].ravel()
        if not v.any():
            per_roi.append((int(b[r]), np.zeros(0, np.int32),
                            np.zeros((0, CELLS * NBINS), np.float32)))
            continue
        shp = valid[r].shape
        bc = lambda a: np.broadcast_to(a, shp).ravel()[v]
        sc = bc(scale[r]).astype(np.float32)
        bn = bc(bins[r]).astype(np.int64)
        cy0 = bc(y0i[r]).astype(np.int64)
        cy1 = bc(y1i[r]).astype(np.int64)
        cx0 = bc(x0i[r]).astype(np.int64)
        cx1 = bc(x1i[r]).astype(np.int64)
        ws = [bc(w00[r]) * sc, bc(w01[r]) * sc,
              bc(w10[r]) * sc, bc(w11[r]) * sc]
        cells = [cy0 * W + cx0, cy0 * W + cx1, cy1 * W + cx0, cy1 * W + cx1]

        cell_all = np.concatenate(cells)
        w_all = np.concatenate(ws).astype(np.float64)
        bin_all = np.concatenate([bn] * 4)

        rows = np.unique(cell_all >> 1).astype(np.int32)        # sorted pairs
        rpos = np.searchsorted(rows, cell_all >> 1)
        key = (rpos * CELLS + (cell_all & 1)) * NBINS + bin_all
        S = np.bincount(key, weights=w_all,
                        minlength=len(rows) * CELLS * NBINS)
        S = S.astype(np.float32).reshape(len(rows), CELLS * NBINS)
        per_roi.append((int(b[r]), rows, S))
    return per_roi


def _pad(n: int) -> int:
    return max(ALIGN, -(-n // ALIGN) * ALIGN)


def _deal_to_cores(per_roi):
    """Assign rois to cores (cores 0-3 image 0, 4-7 image 1) snake-dealt by
    descending padded row count; canonical profile N_v = max over cores of
    the v-th padded count."""
    img_rois = {0: [], 1: []}
    for rid, (img, rows, S) in enumerate(per_roi):
        img_rois[img].append((_pad(len(rows)), rid))
    core_rois = [[] for _ in range(N_CORES)]
    for img, lst in img_rois.items():
        lst.sort(reverse=True)
        cores = list(range(4 * img, 4 * img + 4))
        for i, item in enumerate(lst):
            k = i % 8
            c = cores[k] if k < 4 else cores[7 - k]
            core_rois[c].append(item)
    for c in range(N_CORES):
        core_rois[c].sort(reverse=True)
    K = max(1, max(len(cr) for cr in core_rois))
    N = tuple(max((cr[v][0] if v < len(cr) else 0) for cr in core_rois)
              for v in range(K))
    return core_rois, N


def _segments(N):
    """Canonical layout: roi slot v occupies rows [O_v, O_v + N_v).
    Returns (nslot, per-slot list of (v, a, b), first_slot, last_slot, O)."""
    O = np.concatenate([[0], np.cumsum(N)]).astype(int)
    T = int(O[-1])
    nslot = max(1, -(-T // 128))
    seg = [[] for _ in range(nslot)]
    first_slot = {}
    last_slot = {}
    for v, n in enumerate(N):
        if n == 0:
            continue
        lo, hi = int(O[v]), int(O[v] + n)
        first_slot[v] = lo // 128
        last_slot[v] = (hi - 1) // 128
        for s in range(lo // 128, (hi - 1) // 128 + 1):
            a = max(lo, 128 * s) - 128 * s
            b = min(hi, 128 * (s + 1)) - 128 * s
            seg[s].append((v, a, b))
    return nslot, seg, first_slot, last_slot, O


_PROGRAM_CACHE: dict = {}


def _build_program(N):
    key = N
    if key in _PROGRAM_CACHE:
        return _PROGRAM_CACHE[key]

    from concourse import mybir, bacc
    from concourse.tile import TileContext

    nslot, seg, first_slot, last_slot, _ = _segments(N)
    K = len(N)

    nc = bacc.Bacc("TRN2", target_bir_lowering=False, debug=False,
                   num_devices=N_CORES)
    dataP = nc.declare_dram_parameter("dataP", [NROWSP, ELEM],
                                      mybir.dt.bfloat16, isOutput=False)
    idxs = nc.declare_dram_parameter("idxs", [128, nslot * 8],
                                     mybir.dt.int16, isOutput=False)
    spack = nc.declare_dram_parameter("spack", [128, nslot * CELLS * NBINS],
                                      mybir.dt.bfloat16, isOutput=False)
    out = nc.declare_dram_parameter("out", [NBINS, K * C_FEAT],
                                    mybir.dt.bfloat16, isOutput=True)

    with TileContext(nc) as tc:
        with (
            tc.tile_pool(name="const", bufs=1) as cpool,
            tc.tile_pool(name="gt", bufs=3) as gpool,
            tc.tile_pool(name="ps", bufs=6, space="PSUM") as pspool,
            tc.tile_pool(name="ob", bufs=4) as opool,
        ):
            idx_t = cpool.tile([128, nslot * 8], mybir.dt.int16)
            nc.sync.dma_start(out=idx_t[:], in_=idxs[:])
            s_t = cpool.tile([128, nslot * CELLS * NBINS], mybir.dt.bfloat16)
            # Load S in chunks so early matmuls can start sooner.
            scols = nslot * CELLS * NBINS
            nq = 16
            for q in range(nq):
                lo = q * scols // nq
                hi = (q + 1) * scols // nq
                if hi > lo:
                    nc.sync.dma_start(out=s_t[:, lo:hi], in_=spack[:, lo:hi])

            ps_tiles = {}
            for g0 in range(0, nslot, NSG):
                g = min(NSG, nslot - g0)
                gt = gpool.tile([128, g, ELEM], mybir.dt.bfloat16, name="gt")
                nc.gpsimd.dma_gather(
                    gt[:],
                    dataP[:],
                    idx_t[:, g0 * 8:(g0 + g) * 8],
                    g * 128,
                    g * 128,
                    ELEM,
                )
                for s in range(g0, g0 + g):
                    j = s - g0
                    for (v, a, b) in seg[s]:
                        if v not in ps_tiles:
                            ps_tiles[v] = pspool.tile([NBINS, C_FEAT],
                                                      mybir.dt.float32,
                                                      name="ps")
                        ps = ps_tiles[v]
                        base = s * CELLS * NBINS
                        for e in range(CELLS):
                            nc.tensor.matmul(
                                ps[:],
                                lhsT=s_t[a:b, base + e * NBINS:
                                         base + (e + 1) * NBINS],
                                rhs=gt[a:b, j:j + 1,
                                       e * C_FEAT:(e + 1) * C_FEAT],
                                start=(s == first_slot[v] and e == 0),
                                stop=(s == last_slot[v] and e == CELLS - 1),
                            )
                        if s == last_slot[v]:
                            ob = opool.tile([NBINS, C_FEAT],
                                            mybir.dt.bfloat16, name="ob")
                            if v % 2 == 0:
                                nc.scalar.copy(out=ob[:], in_=ps[:])
                            else:
                                nc.vector.tensor_copy(out=ob[:], in_=ps[:])
                            nc.sync.dma_start(
                                out=out[:, v * C_FEAT:(v + 1) * C_FEAT],
                                in_=ob[:])
                            del ps_tiles[v]
    nc.compile()
    _PROGRAM_CACHE[key] = nc
    return nc


def _core_inputs(per_roi, core_rois, N, dataP_imgs):
    import ml_dtypes

    nslot, _, _, _, O = _segments(N)
    in_maps = []
    roi_of_v = []                      # per core: v -> roi id
    for c in range(N_CORES):
        img = 0 if c < 4 else 1
        idxs = np.zeros((128, nslot * 8), np.int16)
        sp = np.zeros((128, nslot, CELLS * NBINS), np.float32)
        rmap = [-1] * len(N)
        for v, (npad, rid) in enumerate(core_rois[c]):
            rmap[v] = rid
            _, rows, S = per_roi[rid]
            n = len(rows)
            if n == 0:
                continue
            r = int(O[v]) + np.arange(n)
            p, sl = r % 128, r // 128
            # dma_gather idx i (= 128*slot + p within a group) lives at
            # idx tile [i % 16, group_col0 + i // 16] per the interpreter,
            # but HW reads partitions 16..31 (probed); write both. Storing
            # each slot as an 8-column block keeps the mapping valid for
            # any group start.
            idxs[p % 16, sl * 8 + p // 16] = rows
            idxs[16 + p % 16, sl * 8 + p // 16] = rows
            sp[p, sl, :] = S
        in_maps.append({
            "dataP": dataP_imgs[img],
            "idxs": idxs,
            "spack": sp.reshape(128, nslot * CELLS * NBINS)
                       .astype(ml_dtypes.bfloat16),
        })
        roi_of_v.append(rmap)
    return in_maps, roi_of_v


def _prepare(data, rois, offset):
    import ml_dtypes

    data = np.ascontiguousarray(data, dtype=np.float32)
    rois = np.asarray(rois, dtype=np.float32)
    offset = np.asarray(offset, dtype=np.float32)

    per_roi = _host_tables(rois, offset)
    core_rois, N = _deal_to_cores(per_roi)
    nc = _build_program(N)

    # channel-last pair rows per image: [30400, 512] bf16
    dataP_imgs = [
        np.ascontiguousarray(data[i].transpose(1, 2, 0)).reshape(
            NROWSP, ELEM).astype(ml_dtypes.bfloat16)
        for i in range(N_IMG)
    ]
    in_maps, roi_of_v = _core_inputs(per_roi, core_rois, N, dataP_imgs)
    return nc, in_maps, roi_of_v, len(N)


def _collect(results, roi_of_v, K, R):
    out_full = np.zeros((R, C_FEAT, P, P), np.float32)
    for c in range(N_CORES):
        o = np.asarray(results[c]["out"]).astype(np.float32)   # [49, K*256]
        o = o.reshape(NBINS, K, C_FEAT).transpose(1, 2, 0)     # [K, 256, 49]
        for v, rid in enumerate(roi_of_v[c]):
            if rid >= 0:
                out_full[rid] = o[v].reshape(C_FEAT, P, P)
    return out_full


def kernel(data: np.ndarray, rois: np.ndarray, offset: np.ndarray) -> np.ndarray:
    from concourse.bass_utils import run_bass_kernel_spmd

    R = rois.shape[0]
    nc, in_maps, roi_of_v, K = _prepare(data, rois, offset)
    res = run_bass_kernel_spmd(nc, in_maps, list(range(N_CORES)), trace=False)
    return _collect(res.results, roi_of_v, K, R)
